# revision 9
# baseline (speedup 1.0000x reference)
"""DAAGCN Trainium2 kernel — node-sharded across 8 NeuronCores, batch-halves
software pipeline.

Sharding: 1024 graph nodes over 8 cores (128 each), full batch B=64 per core.
Support matrices S_t, Chebyshev S2_t, per-node weights are constant-folded on
the host (they depend only on parameters). The GRU recurrence is independent
per batch element, so each core splits its batch into two halves (X: b 0:32,
Y: b 32:64) and pipelines them half-a-step apart: while one half's AllGather
is in flight the PE works on the other half. Layout changes (node-major <->
feat-major) run on the DMA xbar transpose path instead of the PE.

Per-half layout conventions (per core):
  feat-major: [feat, b*128 + n_local]   (free size 4096, b in half)
  node-major: [n, b*64 + h]             (free size 2048)
"""
import sys

sys.path.insert(0, "/opt/trn_rl_repo")

import numpy as np
import ml_dtypes

import concourse.bass as bass
import concourse.tile as tile
from concourse import bacc, mybir
from concourse.bass_utils import run_bass_kernel_spmd

B, T, N, H, E, KCH, HOR, OD = 64, 12, 1024, 64, 16, 3, 12, 1
NCORES, NL = 8, 128          # cores, nodes per core
BH = B // 2                  # 32 batch per half
FBH = BH * NL                # 4096 feat-major cols per half
FHH = BH * H                 # 2048 node-major cols per half
EPS = 1e-12
BF16 = mybir.dt.bfloat16
F32 = mybir.dt.float32
NPBF = ml_dtypes.bfloat16

_CACHE = {}


# --------------------------------------------------------------------------
# host precompute (weight-only constant folding + x-channel rows)
# --------------------------------------------------------------------------

def _host_precompute(inputs):
    f32 = np.float32
    node_emb = np.asarray(inputs["node_emb"], f32)
    time_emb = np.asarray(inputs["time_emb"], f32)
    src = np.asarray(inputs["source"], f32)[..., 0]            # [B,T,N]

    def ln(x, g, b):
        m = x.mean(-1, keepdims=True)
        v = x.var(-1, keepdims=True)
        return (x - m) / np.sqrt(v + EPS) * g + b

    def support(e):
        logits = e @ e.T
        s = np.exp(logits - logits.max(1, keepdims=True))
        s /= s.sum(1, keepdims=True)
        return s, 2.0 * s @ s - np.eye(N, dtype=f32)

    gate_wp = np.asarray(inputs["gate_wp"], f32)
    upd_wp = np.asarray(inputs["upd_wp"], f32)
    gate_bp = np.asarray(inputs["gate_bp"], f32)
    upd_bp = np.asarray(inputs["upd_bp"], f32)

    names = ["w1g", "w2g", "w1u", "w2u", "stg", "s2tg", "stu", "s2tu", "xr"]
    acc = {c: {k: [] for k in names} for c in range(NCORES)}

    for t in range(T):
        eg = ln(node_emb + time_emb[t][None, :],
                np.asarray(inputs["gate_lng"], f32),
                np.asarray(inputs["gate_lnb"], f32))
        eu = ln(node_emb + time_emb[t][None, :],
                np.asarray(inputs["upd_lng"], f32),
                np.asarray(inputs["upd_lnb"], f32))
        sg, s2g = support(eg)
        su, s2u = support(eu)
        wg = np.einsum("nd,dkio->nkio", eg, gate_wp)           # [N,3,65,128]
        wu = np.einsum("nd,dkio->nkio", eu, upd_wp)            # [N,3,65,64]
        bg = eg @ gate_bp                                      # [N,128]
        bu = eu @ upd_bp                                       # [N,64]
        xt = src[:, t, :]                                      # [B,N]
        xrows = np.stack([xt, xt @ sg.T, xt @ s2g.T,
                          xt, xt @ su.T, xt @ s2u.T], 0)       # [6,B,N]

        for c in range(NCORES):
            lo, hi = c * NL, (c + 1) * NL

            def pack_w(w, bias, O):
                mine = w[lo:hi]                                # [NL,3,65,O]
                # wc (128 rows): conv tile RC = [xg2 feats (k2) ; xg1 (k1)]
                w1 = np.concatenate([mine[:, 2, 1:, :], mine[:, 1, 1:, :]], 1)
                # ws (68 rows): RS = [state/zs (k0 ch1:65) ; x rows ; bias]
                w2 = np.concatenate([mine[:, 0, 1:, :], mine[:, 0, 0:1, :],
                                     mine[:, 1, 0:1, :], mine[:, 2, 0:1, :],
                                     bias[lo:hi][:, None, :]], 1)
                # [NL, rows, O] -> [rows, NL*O]
                return (w1.transpose(1, 0, 2).reshape(128, NL * O),
                        w2.transpose(1, 0, 2).reshape(68, NL * O))

            w1g_, w2g_ = pack_w(wg, bg, 2 * H)
            w1u_, w2u_ = pack_w(wu, bu, H)
            acc[c]["w1g"].append(w1g_); acc[c]["w2g"].append(w2g_)
            acc[c]["w1u"].append(w1u_); acc[c]["w2u"].append(w2u_)

            def pack_s(s):
                # lhsT chunks: [128 (m within chunk), kc*128 + n_local]
                smt = s[lo:hi, :].T                            # [N(m), NL]
                return smt.reshape(8, 128, NL).transpose(1, 0, 2).reshape(128, 8 * NL)

            acc[c]["stg"].append(pack_s(sg)); acc[c]["s2tg"].append(pack_s(s2g))
            acc[c]["stu"].append(pack_s(su)); acc[c]["s2tu"].append(pack_s(s2u))
            acc[c]["xr"].append(xrows[:, :, lo:hi].reshape(6, B * NL))

    per_core = [dict() for _ in range(NCORES)]
    for c in range(NCORES):
        for k in names:
            per_core[c][k] = np.ascontiguousarray(np.stack(acc[c][k]), dtype=NPBF)

    # final-stage constants (same on every core)
    cw = np.asarray(inputs["conv_w"], f32)                     # [12,64]
    g = np.asarray(inputs["out_lng"], f32)
    be = np.asarray(inputs["out_lnb"], f32)
    cb = np.asarray(inputs["conv_b"], f32)
    A = cw * g[None, :]
    fa = np.zeros((64, 14), f32)
    fa[:, :12] = A.T
    fa[:, 12] = 1.0 / 64.0
    fc0 = (-A.sum(1))[None, :].astype(f32)          # [1,12]
    fcc = (cw @ be + cb)[:, None].astype(f32)
    fa = fa.astype(NPBF)
    for c in range(NCORES):
        per_core[c]["fa"] = fa
        per_core[c]["fc0"] = fc0
        per_core[c]["fcc"] = fcc
    return per_core


# --------------------------------------------------------------------------
# device program (identical on all 8 cores; data differs)
# --------------------------------------------------------------------------

def _build_nc():
    nc = bacc.Bacc("TRN2", target_bir_lowering=False, debug=False,
                   num_devices=NCORES)

    def din(name, shape, dt=BF16):
        return nc.dram_tensor(name, shape, dt, kind="ExternalInput").ap()

    w1g_d = din("w1g", [T, 128, NL * 128])
    w2g_d = din("w2g", [T, 68, NL * 128])
    w1u_d = din("w1u", [T, 128, NL * 64])
    w2u_d = din("w2u", [T, 68, NL * 64])
    stg_d = din("stg", [T, 128, 1024])
    s2tg_d = din("s2tg", [T, 128, 1024])
    stu_d = din("stu", [T, 128, 1024])
    s2tu_d = din("s2tu", [T, 128, 1024])
    xr_d = din("xr", [T, 6, B * NL])
    fa_d = din("fa", [64, 14])
    fc0_d = din("fc0", [1, 12], F32)
    fcc_d = din("fcc", [12, 1], F32)
    out_d = nc.dram_tensor("out", [HOR, B * NL], F32, kind="ExternalOutput").ap()

    AF = mybir.ActivationFunctionType
    OP = mybir.AluOpType

    with tile.TileContext(nc) as tc:
        with (
            tc.tile_pool(name="persist", bufs=1) as pp,
            tc.tile_pool(name="wpool", bufs=1) as wp,
            tc.tile_pool(name="spool", bufs=1) as sp,
            tc.tile_pool(name="slices", bufs=3) as slp,
            tc.tile_pool(name="fin", bufs=1) as fin,
            tc.tile_pool(name="convps", bufs=4, space="PSUM") as convps,
            tc.tile_pool(name="pnps", bufs=2, space="PSUM") as pnps,
            tc.tile_pool(name="dram", bufs=4, space="DRAM") as dram,
        ):
            # ---- per-half persistent tiles ----
            Ht, RC, RS, ZR, XGC = ([None, None] for _ in range(5))
            for a in range(2):
                Ht[a] = pp.tile([64, FBH], BF16, tag=f"H{a}", name=f"Ht{a}")
                RC[a] = pp.tile([128, FBH], BF16, tag=f"RC{a}", name=f"RC{a}")
                RS[a] = pp.tile([68, FBH], BF16, tag=f"RS{a}", name=f"RS{a}")
                ZR[a] = pp.tile([128, FBH], BF16, tag=f"ZR{a}", name=f"ZR{a}")
                XGC[a] = pp.tile([128, 2 * FHH], BF16, tag=f"XGC{a}",
                                 name=f"XGC{a}")
                nc.vector.memset(Ht[a][:], 0.0)
                nc.vector.memset(RC[a][:], 0.0)
                nc.gpsimd.memset(RS[a][0:64, :], 0.0)
                nc.gpsimd.memset(RS[a][64:68, :], 1.0)  # row 67 stays ones

            # strided node views: free = b*128+n -> [p, n, b]
            def nb(ap_):
                return ap_.rearrange("p (b n) -> p n b", n=NL)

            RCv = [nb(RC[a][:]) for a in range(2)]
            HC = [RS[a][0:64, :] for a in range(2)]
            RSv = [nb(RS[a][:]) for a in range(2)]
            ZRv = [nb(ZR[a][:]) for a in range(2)]
            HCv = [nb(HC[a]) for a in range(2)]

            cp_v = lambda o, i: nc.vector.tensor_copy(o, i)
            cp_s = lambda o, i: nc.scalar.copy(o, i)
            ce = [cp_v, cp_s]                     # psum-capable copy fns

            def conv_phase(a, st_t, s2t_t, agout):
                """xg1 = (S^T).T @ gathered, xg2 = (S2^T).T @ gathered;
                interleave columns b*128 + (xg2: h | xg1: 64+h) in XGC so one
                xbar transpose per fc chunk lands both in RC feat-major."""
                for fc in range(4):
                    ps1 = convps.tile([128, 512], F32, tag="cps")
                    ps2 = convps.tile([128, 512], F32, tag="cps")
                    for kc in range(8):
                        sl = slp.tile([128, 512], BF16, tag="sl")
                        nc.sync.dma_start(
                            sl[:], agout[kc * 128:(kc + 1) * 128,
                                         fc * 512:(fc + 1) * 512])
                        nc.tensor.matmul(ps1[:], st_t[:, kc * NL:(kc + 1) * NL],
                                         sl[:], start=(kc == 0), stop=(kc == 7))
                        nc.tensor.matmul(ps2[:], s2t_t[:, kc * NL:(kc + 1) * NL],
                                         sl[:], start=(kc == 0), stop=(kc == 7))
                    base = fc * 1024
                    xv = XGC[a][:, base:base + 1024].rearrange(
                        "p (b c) -> p b c", c=128)
                    ce[fc % 2](xv[:, :, 64:128],
                               ps1[:].rearrange("p (b h) -> p b h", h=64))
                    ce[(fc + 1) % 2](xv[:, :, 0:64],
                                     ps2[:].rearrange("p (b h) -> p b h", h=64))
                    nc.sync.dma_start(
                        RC[a][:, base:base + 1024].rearrange(
                            "p (b n) -> p b n", n=NL),
                        XGC[a][:, base:base + 1024], transpose=True)

            def allgather(a, src64):
                """src64 [64, FBH] bf16 feat-major -> gathered [1024, FHH]."""
                nc.sync.dma_start(
                    XGC[a][:, 0:FHH].rearrange("p (b h) -> p b h", h=H),
                    src64, transpose=True)
                agin = dram.tile([128, FHH], BF16, tag="agin")
                agout = dram.tile([1024, FHH], BF16, tag="agout",
                                  addr_space="Shared")
                nc.sync.dma_start(agin[:], XGC[a][:, 0:FHH])
                nc.gpsimd.collective_compute(
                    "AllGather", OP.bypass,
                    replica_groups=[list(range(NCORES))],
                    ins=[agin.opt()], outs=[agout.opt()])
                return agout

            def pernode(a, w1_t, w2_t, O, outv, func, first):
                """per-node matmuls; 8 nodes share one psum bank; fused act."""
                for g0 in range(0, NL, 8):
                    pg = pnps.tile([128, 8 * BH], F32, tag="pn")
                    for j in range(8):
                        n = g0 + j
                        o_sl = pg[0:O, j * BH:(j + 1) * BH]
                        if not first:
                            nc.tensor.matmul(
                                o_sl, w1_t[:, n * O:(n + 1) * O],
                                RCv[a][:, n:n + 1, :], start=True, stop=False)
                        nc.tensor.matmul(
                            o_sl, w2_t[:, n * O:(n + 1) * O],
                            RSv[a][0:68, n:n + 1, :], start=first, stop=True)
                    nc.scalar.activation(
                        outv[0:O, g0:g0 + 8, :],
                        pg[0:O, :].rearrange("p (j b) -> p j b", b=BH), func)

            def load_w(t):
                w1g_t = wp.tile([128, NL * 128], BF16, tag="w1g")
                nc.scalar.dma_start(w1g_t[:], w1g_d[t])
                w2g_t = wp.tile([68, NL * 128], BF16, tag="w2g")
                nc.scalar.dma_start(w2g_t[:], w2g_d[t])
                return w1g_t, w2g_t

            def load_wu(t):
                w1u_t = wp.tile([128, NL * 64], BF16, tag="w1u")
                nc.scalar.dma_start(w1u_t[:], w1u_d[t])
                w2u_t = wp.tile([68, NL * 64], BF16, tag="w2u")
                nc.scalar.dma_start(w2u_t[:], w2u_d[t])
                return w1u_t, w2u_t

            def load_sg(t):
                stg_t = sp.tile([128, 1024], BF16, tag="stg")
                nc.scalar.dma_start(stg_t[:], stg_d[t])
                s2tg_t = sp.tile([128, 1024], BF16, tag="s2tg")
                nc.scalar.dma_start(s2tg_t[:], s2tg_d[t])
                return stg_t, s2tg_t

            def load_su(t):
                stu_t = sp.tile([128, 1024], BF16, tag="stu")
                nc.scalar.dma_start(stu_t[:], stu_d[t])
                s2tu_t = sp.tile([128, 1024], BF16, tag="s2tu")
                nc.scalar.dma_start(s2tu_t[:], s2tu_d[t])
                return stu_t, s2tu_t

            def xr_cols(a):
                return slice(a * FBH, (a + 1) * FBH)

            # steady-state pipeline state
            agout_s = [None, None]
            agout_z = [None, None]

            w2g_t = wp.tile([68, NL * 128], BF16, tag="w2g")
            nc.scalar.dma_start(w2g_t[:], w2g_d[0])
            w1g_t = None
            w2u_t = wp.tile([68, NL * 64], BF16, tag="w2u")
            nc.scalar.dma_start(w2u_t[:], w2u_d[0])
            w1u_t = None
            stg_t = s2tg_t = stu_t = s2tu_t = None

            for t in range(T):
                first = (t == 0)
                # ---------------- gate blocks ----------------
                for a in range(2):
                    nc.gpsimd.dma_start(RS[a][64:67, :],
                                        xr_d[t, 0:3, xr_cols(a)])
                    if not first:
                        conv_phase(a, stg_t, s2tg_t, agout_s[a])
                    pernode(a, w1g_t, w2g_t, 128, ZRv[a], AF.Sigmoid, first)
                    if not first:
                        # zs = z * state -> RS rows 0:64 (bf16)
                        nc.vector.tensor_tensor(RS[a][0:64, :], ZR[a][0:64, :],
                                                Ht[a][:], op=OP.mult)
                        # r down to partitions 0:64 (z dead after zs)
                        nc.gpsimd.dma_start(ZR[a][0:64, :], ZR[a][64:128, :])
                        agout_z[a] = allgather(a, RS[a][0:64, :])
                    else:
                        nc.gpsimd.dma_start(ZR[a][0:64, :], ZR[a][64:128, :])
                    if a == 0 and not first:
                        # prefetch update weights + supports for this step
                        w1u_t, w2u_t = load_wu(t)
                        stu_t, s2tu_t = load_su(t)

                # ---------------- update blocks ----------------
                for a in range(2):
                    nc.gpsimd.dma_start(RS[a][64:67, :],
                                        xr_d[t, 3:6, xr_cols(a)])
                    if not first:
                        conv_phase(a, stu_t, s2tu_t, agout_z[a])
                    pernode(a, w1u_t, w2u_t, 64, HCv[a], AF.Tanh, first)
                    # h = r*(h - hc) + hc, in place (r is at ZR rows 0:64)
                    nc.vector.tensor_tensor(Ht[a][:], Ht[a][:], HC[a][:],
                                            op=OP.subtract)
                    nc.vector.tensor_tensor(Ht[a][:], Ht[a][:], ZR[a][0:64, :],
                                            op=OP.mult)
                    nc.vector.tensor_tensor(Ht[a][:], Ht[a][:], HC[a][:],
                                            op=OP.add)
                    if t < T - 1:
                        nc.vector.tensor_copy(RS[a][0:64, :], Ht[a][:])
                        agout_s[a] = allgather(a, RS[a][0:64, :])
                    if a == 0 and t < T - 1:
                        # prefetch next step's gate weights + supports
                        w1g_t, w2g_t = load_w(t + 1)
                        stg_t, s2tg_t = load_sg(t + 1)

            # ---------------- final LN + end conv ----------------
            FA = pp.tile([64, 14], BF16, tag="FA")
            FC0 = pp.tile([1, 12], F32, tag="FC0")
            ON12 = pp.tile([1, 12], F32, tag="ON12")
            FCC = pp.tile([12, 1], F32, tag="FCC")
            nc.sync.dma_start(FA[:], fa_d[:])
            nc.sync.dma_start(FC0[:], fc0_d[:])
            nc.vector.memset(ON12[:], 1.0)
            nc.sync.dma_start(FCC[:], fcc_d[:])

            for ch in range(16):
                a, fc = ch // 8, ch % 8
                sl_ = slice(fc * 512, (fc + 1) * 512)
                osl = slice(a * FBH + fc * 512, a * FBH + (fc + 1) * 512)
                sq = fin.tile([64, 512], BF16, tag="fsq")
                nc.scalar.activation(sq[:], Ht[a][:, sl_], AF.Square)
                psA = convps.tile([12, 512], F32, tag="cps")
                nc.tensor.matmul(psA[:], FA[:, 0:12], Ht[a][:, sl_],
                                 start=True, stop=True)
                psM = convps.tile([1, 512], F32, tag="cps")
                nc.tensor.matmul(psM[:], FA[:, 12:13], Ht[a][:, sl_],
                                 start=True, stop=True)
                psB = convps.tile([1, 512], F32, tag="cps")
                nc.tensor.matmul(psB[:], FA[:, 12:13], sq[:],
                                 start=True, stop=True)
                sA = fin.tile([12, 512], F32, tag="fsA")
                nc.vector.tensor_copy(sA[:], psA[:])
                sM = fin.tile([1, 512], F32, tag="fsM")
                nc.scalar.copy(sM[:], psM[:])
                sM2 = fin.tile([1, 512], F32, tag="fsM2")
                nc.scalar.copy(sM2[:], psB[:])
                v = fin.tile([1, 512], F32, tag="fv")
                nc.vector.tensor_tensor(v[:], sM[:], sM[:], op=OP.mult)
                nc.vector.tensor_tensor(v[:], sM2[:], v[:], op=OP.subtract)
                nc.vector.tensor_scalar_add(v[:], v[:], EPS)
                sd = fin.tile([1, 512], F32, tag="fsd")
                nc.scalar.activation(sd[:], v[:], AF.Sqrt)
                nc.vector.reciprocal(v[:], sd[:])          # v = rstd
                mr = fin.tile([1, 512], F32, tag="fmr")
                nc.vector.tensor_tensor(mr[:], sM[:], v[:], op=OP.mult)
                psR = convps.tile([12, 512], F32, tag="cps")
                nc.tensor.matmul(psR[:], ON12[:], v[:], start=True, stop=True)
                psM2 = convps.tile([12, 512], F32, tag="cps")
                nc.tensor.matmul(psM2[:], FC0[:], mr[:], start=True, stop=True)
                och = fin.tile([12, 512], F32, tag="foch")
                nc.vector.tensor_tensor(och[:], sA[:], psR[:], op=OP.mult)
                nc.vector.tensor_tensor(och[:], och[:], psM2[:], op=OP.add)
                nc.vector.tensor_scalar_add(och[:], och[:], FCC[:, 0:1])
                nc.gpsimd.dma_start(out_d[:, osl], och[:])

    nc.compile()
    return nc


# --------------------------------------------------------------------------
# entry point
# --------------------------------------------------------------------------

def kernel(**inputs) -> np.ndarray:
    per_core = _host_precompute(inputs)
    if "nc" not in _CACHE:
        _CACHE["nc"] = _build_nc()
    res = run_bass_kernel_spmd(_CACHE["nc"], per_core, list(range(NCORES)))
    full = np.zeros((B, HOR, N, OD), np.float32)
    for c in range(NCORES):
        co = np.asarray(res.results[c]["out"], np.float32).reshape(HOR, B, NL)
        full[:, :, c * NL:(c + 1) * NL, 0] = co.transpose(1, 0, 2)
    return full
